# revision 67
# baseline (speedup 1.0000x reference)
"""Multi-Head Latent Attention (MLA) Trainium2 kernel, 8-core SPMD.

Sharding: core c -> batch b = c // 4, head-group g = c % 4 (4 heads each).
The latent path is S-sharded: core g owns global rows [512g, 512(g+1)).
Latents are produced DIRECTLY TRANSPOSED on the PE (stationary = weight
tile, moving = xT) so the AllGather staging writes are fat contiguous
DMAs and consumers do plain block-copy gathers (descriptor generation on
the HWDGE engines is ~5ns/descriptor, so every DMA keeps >=1KB per
partition line).  RMSNorm in the transposed layout: sum-of-squares via a
ones-matmul over Squared tiles, rsqrt of row 0, broadcast back to 128
partitions with a rank-1 matmul, and the scale fused into the PSUM
eviction multiply.

Collective pipeline per 4-core group: kv-AG -> q-AG(qr 0:768) ->
q-AG(qr 768:1536) -> A2A(heads01) -> A2A(heads23); the q decompress
accumulation chains pause mid-PSUM on the second q-AG's semaphore, so
half the contraction overlaps the collective.  The output projection is
ROW-parallel: attention outputs are exchanged with two 8-rank AllToAlls
(each core keeps 256 rows of each batch) and projected with the full
replicated w_proj; accumulation over the two A2A chunks via SBUF f32.

Math notes:
 - All matmul operands are bf16 (fp32 PSUM accumulate); rope pair-dims are
   permuted (even dims first) so the rotation works on contiguous 32-blocks,
   applied identically to q and k so dot products are unchanged.
 - Odd heads inside a pair use a half-swapped partition layout
   ([rope | nope] instead of [nope | rope]) in both qT and kT so every PSUM
   eviction is partition-aligned.  Dot products are unaffected.
 - q_norm_w / kv_norm_w are folded into the decompress weights on the host.
"""

import sys

for _p in ("/opt/trn_rl_repo", "/opt/pypackages"):
    if _p not in sys.path:
        sys.path.append(_p)

import numpy as np
import ml_dtypes

B, S, D = 2, 2048, 2048
H, HD, RD, ND = 16, 128, 64, 64
QR, KVR = 1536, 512
EPS = 1e-6
G = 4            # cores per batch group
NC = 8
HC = H // G      # heads per core
SC = S // G      # latent-path S rows per core (contiguous block)
NT = S // 128    # 16 s-tiles
NW = S // 512    # 4  sq windows
KVW = KVR + RD   # 576
SCALE = 1.0 / float(np.sqrt(HD))
NEG = -30000.0   # additive mask; * SCALE stays << exp underflow

BF = ml_dtypes.bfloat16

_cached = {}


def _build():
    import concourse.bass as bass
    import concourse.mybir as mybir
    import concourse.tile as tile
    from concourse import bacc
    from concourse.masks import make_identity
    from contextlib import ExitStack

    f32 = mybir.dt.float32
    bf16 = mybir.dt.bfloat16
    AF = mybir.ActivationFunctionType

    nc = bacc.Bacc()

    # ---- parameters (per-core host-prepped, 128-row PACKED) ----
    P_xTp = nc.declare_dram_parameter("xTp", [128, 16 * SC], bf16, isOutput=False)
    P_wkvp = nc.declare_dram_parameter("wkvp", [128, 16 * KVR], bf16, isOutput=False)
    P_wkrp = nc.declare_dram_parameter("wkrp", [128, 16 * RD], bf16, isOutput=False)
    P_wlqp = nc.declare_dram_parameter("wlqp", [128, 16 * QR], bf16, isOutput=False)
    P_wdqn = nc.declare_dram_parameter("wdqn", [128, 12 * HC * ND], bf16, isOutput=False)
    P_wdqr = nc.declare_dram_parameter("wdqr", [128, 12 * HC * RD], bf16, isOutput=False)
    P_wdkn = nc.declare_dram_parameter("wdkn", [128, 4 * HC * ND], bf16, isOutput=False)
    P_wdv = nc.declare_dram_parameter("wdv", [128, 4 * HC * HD], bf16, isOutput=False)
    P_wproj = nc.declare_dram_parameter("wproj", [H * HD, H * HD], bf16,
                                        isOutput=False)
    # rope tables; A variant = [cos|sin]*HC, B = [sin|cos]*HC, packed by s-tile
    P_csA = nc.declare_dram_parameter("csA", [128, NT * HC * RD], bf16, isOutput=False)
    P_csB = nc.declare_dram_parameter("csB", [128, NT * HC * RD], bf16, isOutput=False)
    P_csAc = nc.declare_dram_parameter("csAc", [128, 4 * RD], bf16, isOutput=False)
    P_csBc = nc.declare_dram_parameter("csBc", [128, 4 * RD], bf16, isOutput=False)
    P_mask = nc.declare_dram_parameter("maskT", [128, 128], f32, isOutput=False)
    P_out = nc.declare_dram_parameter("out", [SC, H * HD], f32, isOutput=True)

    groups = [[0, 1, 2, 3], [4, 5, 6, 7]]
    groups8 = [[0, 1, 2, 3, 4, 5, 6, 7]]

    with ExitStack() as top:
        tc = top.enter_context(tile.TileContext(nc))

        dram = top.enter_context(tc.tile_pool(name="dram", bufs=1, space="DRAM"))
        gkv_in = dram.tile([KVW, SC], bf16, tag="gkv_in", name="gkv_in")
        gkv_out = dram.tile([G, KVW, SC], bf16, tag="gkv_out", name="gkv_out")
        gq_in = [dram.tile([768, SC], bf16, tag=f"gq_in{j}", name=f"gq_in{j}")
                 for j in range(2)]
        gq_out = [dram.tile([G, 768, SC], bf16, tag=f"gq_out{j}", name=f"gq_out{j}")
                  for j in range(2)]
        a2a_in = [dram.tile([NC, 2 * HD, 256], bf16, tag=f"a2a_in{i}",
                            name=f"a2a_in{i}") for i in range(2)]
        a2a_out = [dram.tile([NC, 2 * HD, 256], bf16, tag=f"a2a_out{i}",
                             name=f"a2a_out{i}") for i in range(2)]

        const = top.enter_context(tc.tile_pool(name="const", bufs=1))
        ident = const.tile([128, 128], bf16, tag="ident", name="ident")
        make_identity(nc, ident)
        ones_sb = const.tile([128, 128], bf16, tag="ones", name="ones")
        nc.vector.memset(ones_sb[:], 1.0)
        onecol = const.tile([1, 128], bf16, tag="onecol", name="onecol")
        nc.vector.memset(onecol[:], 1.0)
        mask_sb = const.tile([128, 128], f32, tag="mask", name="mask")
        nc.sync.dma_start(mask_sb[:], P_mask[:])
        eps_row = const.tile([1, 1], f32, tag="eps_row", name="eps_row")
        nc.vector.memset(eps_row[:], EPS)

        persist = top.enter_context(tc.tile_pool(name="persist", bufs=1))
        qT = [persist.tile([128, S], bf16, tag=f"qT{h}", name=f"qT{h}")
              for h in range(HC)]
        kT = [persist.tile([128, S], bf16, tag=f"kT{h}", name=f"kT{h}")
              for h in range(HC)]
        v_sb = [persist.tile([128, HC * HD], bf16, tag=f"v{t}", name=f"v{t}")
                for t in range(NT)]

        # decompress weights + rope tables (behind the latent inputs on the
        # sync queue; freed after q decompress)
        ctx_pcS = ExitStack()
        pcS = ctx_pcS.enter_context(tc.tile_pool(name="pcS", bufs=1))
        wdqn_sb = pcS.tile([128, 12 * HC * ND], bf16, tag="wdqn", name="wdqn")
        wdqr_sb = pcS.tile([128, 12 * HC * RD], bf16, tag="wdqr", name="wdqr")
        wdkn_sb = pcS.tile([128, 4 * HC * ND], bf16, tag="wdkn", name="wdkn")
        wdv_sb = pcS.tile([128, 4 * HC * HD], bf16, tag="wdv", name="wdv")
        csA_sb = pcS.tile([128, NT * HC * RD], bf16, tag="csA", name="csA")
        csB_sb = pcS.tile([128, NT * HC * RD], bf16, tag="csB", name="csB")

        # ================= Phase A: transposed latents on own rows ==========
        ctxA = ExitStack()
        pa = ctxA.enter_context(tc.tile_pool(name="pa", bufs=1))
        pa_mv = ctxA.enter_context(tc.tile_pool(name="pa_mv", bufs=2))
        pa_q = ctxA.enter_context(tc.tile_pool(name="pa_q", bufs=1))
        pa_ps = ctxA.enter_context(
            tc.tile_pool(name="pa_ps", bufs=4, space="PSUM"))
        pa_nm = ctxA.enter_context(
            tc.tile_pool(name="pa_nm", bufs=1, space="PSUM"))

        xTp = pa.tile([128, 16 * SC], bf16, tag="xTp", name="xTp")
        nc.sync.dma_start(xTp[:], P_xTp[:])
        wkvp = pa.tile([128, 16 * KVR], bf16, tag="wkvp", name="wkvp")
        nc.sync.dma_start(wkvp[:], P_wkvp[:])
        wkrp = pa.tile([128, 16 * RD], bf16, tag="wkrp", name="wkrp")
        nc.sync.dma_start(wkrp[:], P_wkrp[:])
        csAcp = pa.tile([128, 4 * RD], bf16, tag="csAcp", name="csAcp")
        nc.sync.dma_start(csAcp[:], P_csAc[:])
        csBcp = pa.tile([128, 4 * RD], bf16, tag="csBcp", name="csBcp")
        nc.sync.dma_start(csBcp[:], P_csBc[:])
        wlqp = pa.tile([128, 16 * QR], bf16, tag="wlqp", name="wlqp")
        nc.sync.dma_start(wlqp[:], P_wlqp[:])
        nc.sync.dma_start(wdkn_sb[:], P_wdkn[:])
        nc.sync.dma_start(wdv_sb[:], P_wdv[:])
        nc.sync.dma_start(wdqn_sb[:], P_wdqn[:])
        nc.sync.dma_start(wdqr_sb[:], P_wdqr[:])
        nc.sync.dma_start(csA_sb[:], P_csA[:])
        nc.sync.dma_start(csB_sb[:], P_csB[:])

        # ---- PASS K: kv (transposed) + krope (natural) ----
        pkv = []
        for m in range(4):
            p = pa_ps.tile([128, SC], f32, tag="lat_ps", name="lat_ps")
            for dt_ in range(16):
                nc.tensor.matmul(
                    p[:], wkvp[:, dt_ * KVR + m * 128:dt_ * KVR + (m + 1) * 128],
                    xTp[:, dt_ * SC:(dt_ + 1) * SC],
                    start=dt_ == 0, stop=dt_ == 15)
            pkv.append(p)
        # sum of squares over kv dims -> per-s norm factors (row 0)
        nrm = pa_nm.tile([128, SC], f32, tag="nrm_ps", name="nrm_ps")
        for m in range(4):
            sq = pa_mv.tile([128, SC], bf16, tag="sqt", name="sqt")
            nc.scalar.activation(sq[:], pkv[m][:], AF.Square)
            nc.tensor.matmul(nrm[:], ones_sb[:], sq[:],
                             start=m == 0, stop=m == 3)
        rrow = pa_mv.tile([1, SC], f32, tag="rrow", name="rrow")
        nc.scalar.activation(rrow[:], nrm[0:1, :], AF.Sqrt,
                             bias=eps_row[:], scale=1.0 / KVR)
        nc.vector.reciprocal(rrow[:], rrow[:])
        rbf = pa_mv.tile([1, SC], bf16, tag="rbf", name="rbf")
        nc.vector.tensor_copy(rbf[:], rrow[:])
        bc_ps = pa_nm.tile([128, SC], f32, tag="bc_ps", name="bc_ps")
        nc.tensor.matmul(bc_ps[:], onecol[:], rbf[:], start=True, stop=True)
        bc_sb = pa_mv.tile([128, SC], f32, tag="bc_sb", name="bc_sb")
        nc.scalar.copy(bc_sb[:], bc_ps[:])
        for m in range(4):
            ev = pa_mv.tile([128, SC], bf16, tag="ev", name="ev")
            nc.vector.tensor_mul(ev[:], pkv[m][:], bc_sb[:])
            nc.scalar.dma_start(gkv_in[m * 128:(m + 1) * 128, :], ev[:])

        # krope: natural layout, rotate, PE-transpose, stage
        for j in range(4):
            pkr = pa_ps.tile([128, RD], f32, tag="lat_ps", name="lat_ps")
            for dt_ in range(16):
                nc.tensor.matmul(
                    pkr[:], xTp[:, dt_ * SC + j * 128:dt_ * SC + (j + 1) * 128],
                    wkrp[:, dt_ * RD:(dt_ + 1) * RD],
                    start=dt_ == 0, stop=dt_ == 15)
            kr_raw = pa_mv.tile([128, RD], bf16, tag="kr_raw", name="kr_raw")
            nc.scalar.copy(kr_raw[:], pkr[:])
            pr1 = pa_mv.tile([128, RD], bf16, tag="pr1", name="pr1")
            pr2 = pa_mv.tile([128, RD], bf16, tag="pr2", name="pr2")
            nc.vector.tensor_mul(pr1[:], kr_raw[:],
                                 csAcp[:, j * RD:(j + 1) * RD])
            nc.vector.tensor_mul(pr2[:], kr_raw[:],
                                 csBcp[:, j * RD:(j + 1) * RD])
            krot = pa_mv.tile([128, RD], bf16, tag="krot", name="krot")
            nc.vector.tensor_sub(krot[:, 0:32], pr1[:, 0:32], pr1[:, 32:64])
            nc.vector.tensor_add(krot[:, 32:64], pr2[:, 0:32], pr2[:, 32:64])
            tp = pa_nm.tile([128, 128], bf16, tag="krtp", name="krtp")
            nc.tensor.transpose(tp[0:64, :], krot[:], ident[:])
            tps = pa_mv.tile([64, 128], bf16, tag="krtps", name="krtps")
            nc.scalar.copy(tps[:], tp[0:64, :])
            nc.scalar.dma_start(
                gkv_in[KVR:KVW, j * 128:(j + 1) * 128], tps[:])

        nc.gpsimd.collective_compute(
            "AllGather", mybir.AluOpType.bypass,
            replica_groups=groups,
            ins=[gkv_in.opt()], outs=[gkv_out.opt()])

        # ---- PASS Q: cq transposed; norm accumulated across all 12 tiles ----
        praw = []
        nrmq = pa_nm.tile([128, SC], f32, tag="nrm_ps", name="nrm_ps")
        for mq in range(12):
            p = pa_ps.tile([128, SC], f32, tag="lat_ps", name="lat_ps")
            for dt_ in range(16):
                nc.tensor.matmul(
                    p[:],
                    wlqp[:, dt_ * QR + mq * 128:dt_ * QR + (mq + 1) * 128],
                    xTp[:, dt_ * SC:(dt_ + 1) * SC],
                    start=dt_ == 0, stop=dt_ == 15)
            sq = pa_mv.tile([128, SC], bf16, tag="sqt", name="sqt")
            nc.scalar.activation(sq[:], p[:], AF.Square)
            raw = pa_q.tile([128, SC], bf16, tag=f"qraw{mq}", name=f"qraw{mq}")
            nc.scalar.copy(raw[:], p[:])
            praw.append(raw)
            nc.tensor.matmul(nrmq[:], ones_sb[:], sq[:],
                             start=mq == 0, stop=mq == 11,
                             skip_group_check=True)
        rq = pa_mv.tile([1, SC], f32, tag="rrow", name="rrow")
        nc.scalar.activation(rq[:], nrmq[0:1, :], AF.Sqrt,
                             bias=eps_row[:], scale=1.0 / QR)
        nc.vector.reciprocal(rq[:], rq[:])
        rqbf = pa_mv.tile([1, SC], bf16, tag="rbf", name="rbf")
        nc.vector.tensor_copy(rqbf[:], rq[:])
        bq_ps = pa_nm.tile([128, SC], f32, tag="bc_ps", name="bc_ps")
        nc.tensor.matmul(bq_ps[:], onecol[:], rqbf[:], start=True, stop=True)
        bq_sb = pa_mv.tile([128, SC], f32, tag="bc_sb", name="bc_sb")
        nc.scalar.copy(bq_sb[:], bq_ps[:])

        # scale + stage, AG per qr-half
        for half in range(2):
            for m in range(6):
                raw = praw[6 * half + m]
                nc.vector.tensor_mul(raw[:], raw[:], bq_sb[:])
                nc.scalar.dma_start(
                    gq_in[half][m * 128:(m + 1) * 128, :], raw[:])
            nc.gpsimd.collective_compute(
                "AllGather", mybir.AluOpType.bypass,
                replica_groups=groups,
                ins=[gq_in[half].opt()], outs=[gq_out[half].opt()])

        ctxA.close()

        # ================= Phase C: decompress q/k/v =================
        ctx_pcG = ExitStack()
        pcG = ctx_pcG.enter_context(tc.tile_pool(name="pcG", bufs=1))

        ctxC = ExitStack()
        pc_mv = ctxC.enter_context(tc.tile_pool(name="pc_mv", bufs=4))
        ctxCp = ExitStack()
        pc_ps = ctxCp.enter_context(
            tc.tile_pool(name="pc_ps", bufs=4, space="PSUM"))
        pc_tp = ctxCp.enter_context(
            tc.tile_pool(name="pc_tp", bufs=2, space="PSUM"))

        # gathered latents -> SBUF, plain block copies (1KB lines)
        nkvT = []
        for rt in range(KVR // 128):
            t = pcG.tile([128, S], bf16, tag=f"nkvT{rt}", name=f"nkvT{rt}")
            for g2 in range(G):
                nc.scalar.dma_start(
                    t[:, g2 * SC:(g2 + 1) * SC],
                    gkv_out[g2, rt * 128:(rt + 1) * 128, :])
            nkvT.append(t)
        # shared (already rotated) q-rope -> directly into qT[h] rope slot
        for h in range(HC):
            roff = 64 if h % 2 == 0 else 0   # even: [nope|rope], odd: [rope|nope]
            for g2 in range(G):
                nc.scalar.dma_start(
                    qT[h][roff:roff + 64, g2 * SC:(g2 + 1) * SC],
                    gkv_out[g2, KVR:KVW, :])

        # ---- v (natural layout) ----
        for st in range(NT):
            ps = pc_ps.tile([128, HC * HD], f32, tag="dec_ps", name="dec_ps")
            for rt in range(KVR // 128):
                nc.tensor.matmul(
                    ps[:], nkvT[rt][:, st * 128:(st + 1) * 128],
                    wdv_sb[:, rt * HC * HD:(rt + 1) * HC * HD],
                    start=rt == 0, stop=rt == KVR // 128 - 1)
            nc.vector.tensor_copy(v_sb[st][:], ps[:])

        # ---- k_nope: head-pair packed, transposed layout ----
        for p in range(HC // 2):
            psl = [pc_ps.tile([128, 512], f32, tag="dec_ps", name="dec_ps")
                   for _ in range(S // 512)]
            for rt in range(KVR // 128):
                stat = wdkn_sb[:, rt * HC * ND + p * 128:
                               rt * HC * ND + (p + 1) * 128]
                for sc4 in range(S // 512):
                    nc.tensor.matmul(
                        psl[sc4][:], stat,
                        nkvT[rt][:, sc4 * 512:(sc4 + 1) * 512],
                        start=rt == 0, stop=rt == KVR // 128 - 1)
            h0, h1 = 2 * p, 2 * p + 1
            for sc4 in range(S // 512):
                sl = slice(sc4 * 512, (sc4 + 1) * 512)
                nc.vector.tensor_copy(kT[h0][0:64, sl], psl[sc4][0:64, :])
                nc.vector.tensor_copy(kT[h1][64:128, sl], psl[sc4][64:128, :])

        # gathered q latents: qr 0:768 on scalar queue, 768:1536 on sync
        # (separate queues so the second AG's wait can't block the first's)
        nqT = []
        for rt in range(QR // 128):
            t = pcG.tile([128, S], bf16, tag=f"nqT{rt}", name=f"nqT{rt}")
            nqT.append(t)
        for rt in range(6):
            for g2 in range(G):
                nc.scalar.dma_start(
                    nqT[rt][:, g2 * SC:(g2 + 1) * SC],
                    gq_out[0][g2, rt * 128:(rt + 1) * 128, :])
        for rt in range(6, 12):
            for g2 in range(G):
                nc.sync.dma_start(
                    nqT[rt][:, g2 * SC:(g2 + 1) * SC],
                    gq_out[1][g2, (rt - 6) * 128:(rt - 5) * 128, :])

        # ---- q_nope: head-pair packed; chains pause at rt=6 until q-AG B ----
        for p in range(HC // 2):
            psl = [pc_ps.tile([128, 512], f32, tag="dec_ps", name="dec_ps")
                   for _ in range(S // 512)]
            for rt in range(QR // 128):
                stat = wdqn_sb[:, rt * HC * ND + p * 128:
                               rt * HC * ND + (p + 1) * 128]
                for sc4 in range(S // 512):
                    nc.tensor.matmul(
                        psl[sc4][:], stat,
                        nqT[rt][:, sc4 * 512:(sc4 + 1) * 512],
                        start=rt == 0, stop=rt == QR // 128 - 1)
            h0, h1 = 2 * p, 2 * p + 1
            for sc4 in range(S // 512):
                sl = slice(sc4 * 512, (sc4 + 1) * 512)
                nc.vector.tensor_copy(qT[h0][0:64, sl], psl[sc4][0:64, :])
                nc.vector.tensor_copy(qT[h1][64:128, sl], psl[sc4][64:128, :])

        # ---- q_rope natural, rotate, transpose into kT rope slots ----
        for st in range(NT):
            ps = pc_ps.tile([128, HC * RD], f32, tag="dec_ps", name="dec_ps")
            for rt in range(QR // 128):
                nc.tensor.matmul(
                    ps[:], nqT[rt][:, st * 128:(st + 1) * 128],
                    wdqr_sb[:, rt * HC * RD:(rt + 1) * HC * RD],
                    start=rt == 0, stop=rt == QR // 128 - 1)
            qr_sb = pc_mv.tile([128, HC * RD], bf16, tag="qr_sb", name="qr_sb")
            nc.scalar.copy(qr_sb[:], ps[:])
            pr1 = pc_mv.tile([128, HC * RD], bf16, tag="qpr1", name="qpr1")
            pr2 = pc_mv.tile([128, HC * RD], bf16, tag="qpr2", name="qpr2")
            csl = slice(st * HC * RD, (st + 1) * HC * RD)
            nc.vector.tensor_mul(pr1[:], qr_sb[:], csA_sb[:, csl])
            nc.vector.tensor_mul(pr2[:], qr_sb[:], csB_sb[:, csl])
            rot = pc_mv.tile([128, HC * RD], bf16, tag="qrot", name="qrot")
            r3a = rot[:].rearrange("p (h two f) -> p h two f", two=2, f=32)
            p3a = pr1[:].rearrange("p (h two f) -> p h two f", two=2, f=32)
            p3b = pr2[:].rearrange("p (h two f) -> p h two f", two=2, f=32)
            nc.vector.tensor_sub(r3a[:, :, 0, :], p3a[:, :, 0, :], p3a[:, :, 1, :])
            nc.vector.tensor_add(r3a[:, :, 1, :], p3b[:, :, 0, :], p3b[:, :, 1, :])
            for h in range(HC):
                tp = pc_tp.tile([128, 128], bf16, tag="ktp", name="ktp")
                roff = 64 if h % 2 == 0 else 0
                nc.tensor.transpose(
                    tp[roff:roff + 64, :],
                    rot[:, h * RD:(h + 1) * RD], ident[:])
                nc.scalar.copy(
                    kT[h][roff:roff + 64, st * 128:(st + 1) * 128],
                    tp[roff:roff + 64, :])

        ctxCp.close()
        ctxC.close()
        ctx_pcG.close()
        ctx_pcS.close()

        # projection weights (SBUF freed by the closes above)
        pe = top.enter_context(tc.tile_pool(name="pe", bufs=1))
        wpj = []
        for ot in range(H * HD // 128):
            t = pe.tile([128, H * HD], bf16, tag=f"wpj{ot}", name=f"wpj{ot}")
            nc.sync.dma_start(t[:], P_wproj[ot * 128:(ot + 1) * 128, :])
            wpj.append(t)

        # ================= Phase D: causal SDPA =================
        ctxD = ExitStack()
        pd_mv = ctxD.enter_context(tc.tile_pool(name="pd_mv", bufs=4))
        pd_probs = ctxD.enter_context(tc.tile_pool(name="pd_probs", bufs=6))
        pd_sc = ctxD.enter_context(
            tc.tile_pool(name="pd_sc", bufs=4, space="PSUM"))
        pd_acc = ctxD.enter_context(
            tc.tile_pool(name="pd_acc", bufs=2, space="PSUM"))

        def sdpa_block(h, w):
            vcol = slice(h * HD, (h + 1) * HD)
            nk = 4 * (w + 1)
            den = pd_acc.tile([128, 512], f32, tag="den", name="den")
            att = pd_acc.tile([128, 512], f32, tag="att", name="att")
            for kt in range(nk):
                off = max(0, 128 * kt - 512 * w)
                sq0 = 512 * w + off
                ssc = pd_sc.tile([128, 512], f32, tag="ssc", name="ssc")
                nc.tensor.matmul(
                    ssc[:, off:512],
                    kT[h][:, kt * 128:(kt + 1) * 128],
                    qT[h][:, sq0:512 * (w + 1)],
                    start=True, stop=True)
                if kt >= 4 * w:   # block containing the diagonal
                    nc.vector.tensor_add(
                        ssc[:, off:off + 128],
                        ssc[:, off:off + 128], mask_sb[:])
                probs = pd_probs.tile([128, 512], bf16, tag="probs", name="probs")
                nc.scalar.activation(
                    probs[:, off:512], ssc[:, off:512],
                    AF.Exp, scale=SCALE)
                nc.tensor.matmul(
                    den[:, off:512], ones_sb[:], probs[:, off:512],
                    start=kt == 0, stop=kt == nk - 1)
                nc.tensor.matmul(
                    att[:, off:512], v_sb[kt][:, vcol],
                    probs[:, off:512],
                    start=kt == 0, stop=kt == nk - 1)
            rec = pd_mv.tile([128, 512], f32, tag="rec", name="rec")
            nc.vector.reciprocal(rec[:], den[:])
            outT = pd_mv.tile([128, 512], bf16, tag="outT", name="outT")
            nc.vector.tensor_mul(outT[:], att[:], rec[:])
            # window w (sq rows [512w, 512w+512) of this batch) feeds the two
            # A2A dest shards 2w and 2w+1 (256 rows each)
            hb = (h % 2) * 128
            nc.sync.dma_start(
                a2a_in[h // 2][2 * w, hb:hb + 128, :], outT[:, 0:256])
            nc.sync.dma_start(
                a2a_in[h // 2][2 * w + 1, hb:hb + 128, :], outT[:, 256:512])

        # head-major; A2A after h1 and after h3
        a_sb = [[None] * (H * HD // 128) for _ in range(2)]
        for h in range(HC):
            for w in range(NW):
                sdpa_block(h, w)
            if h == 1 or h == 3:
                i = h // 2
                nc.gpsimd.collective_compute(
                    "AllToAll", mybir.AluOpType.bypass,
                    replica_groups=groups8,
                    ins=[a2a_in[i].opt()], outs=[a2a_out[i].opt()])
                # gpsimd queue: it already serializes on the A2A completion,
                # so these loads never block sync/scalar work
                for src in range(NC):
                    for k in range(2):
                        ot = 4 * (src % G) + 2 * i + k
                        t = pe.tile([128, 256], bf16, tag=f"aT{src}_{2*i+k}",
                                    name=f"aT{src}_{2*i+k}")
                        nc.gpsimd.dma_start(
                            t[:], a2a_out[i][src, k * 128:(k + 1) * 128, :])
                        a_sb[src // G][ot] = t
        ctxD.close()

        # ===== row-parallel projection (accumulate over the 2 A2A chunks) ====
        pe_mv = top.enter_context(tc.tile_pool(name="pe_mv", bufs=4))
        pe_acc = top.enter_context(tc.tile_pool(name="pe_acc", bufs=1))
        pe_ps = top.enter_context(
            tc.tile_pool(name="pe_ps", bufs=2, space="PSUM"))

        OT1 = [4 * g2 + k for g2 in range(G) for k in (0, 1)]
        OT2 = [4 * g2 + k for g2 in range(G) for k in (2, 3)]
        acc_sb = [pe_acc.tile([128, 512], f32, tag=f"pacc{i}", name=f"pacc{i}")
                  for i in range(16)]
        # P_out rows: [0:256) = this core's batch-0 rows, [256:512) = batch-1
        for half in range(2):
            for st2 in range(2):
                for cb in range(4):
                    ps = pe_ps.tile([128, 512], f32, tag="proj_ps",
                                    name="proj_ps")
                    for i, ot in enumerate(OT1):
                        nc.tensor.matmul(
                            ps[:],
                            a_sb[half][ot][:, st2 * 128:(st2 + 1) * 128],
                            wpj[ot][:, cb * 512:(cb + 1) * 512],
                            start=i == 0, stop=i == len(OT1) - 1)
                    nc.scalar.copy(acc_sb[8 * half + 4 * st2 + cb][:], ps[:])
        for half in range(2):
            for st2 in range(2):
                for cb in range(4):
                    ps = pe_ps.tile([128, 512], f32, tag="proj_ps",
                                    name="proj_ps")
                    for i, ot in enumerate(OT2):
                        nc.tensor.matmul(
                            ps[:],
                            a_sb[half][ot][:, st2 * 128:(st2 + 1) * 128],
                            wpj[ot][:, cb * 512:(cb + 1) * 512],
                            start=i == 0, stop=i == len(OT2) - 1)
                    o_sb = pe_mv.tile([128, 512], f32, tag="o_sb", name="o_sb")
                    nc.vector.tensor_add(
                        o_sb[:], ps[:], acc_sb[8 * half + 4 * st2 + cb][:])
                    r0 = half * 256 + st2 * 128
                    nc.sync.dma_start(
                        P_out[r0:r0 + 128, cb * 512:(cb + 1) * 512], o_sb[:])

    nc.compile()
    return nc


def _get_nc():
    if "nc" not in _cached:
        _cached["nc"] = _build()
    return _cached["nc"]


def _pack(m):
    """[R, C] -> [128, (R//128)*C] with 128-row tiles laid side by side."""
    r, c = m.shape
    return np.ascontiguousarray(
        m.reshape(r // 128, 128, c).transpose(1, 0, 2).reshape(128, -1))


def _prep_inputs(inputs):
    x = np.asarray(inputs["x"], np.float32)
    fc = np.asarray(inputs["freqs_cos"], np.float32)   # [S, 32]
    fs = np.asarray(inputs["freqs_sin"], np.float32)
    w_cq = np.asarray(inputs["w_cq"], np.float32)
    w_dq_nope = np.asarray(inputs["w_dq_nope"], np.float32)
    w_dq_rope = np.asarray(inputs["w_dq_rope"], np.float32)
    w_ckv = np.asarray(inputs["w_ckv"], np.float32)
    w_dk_nope = np.asarray(inputs["w_dk_nope"], np.float32)
    w_dv = np.asarray(inputs["w_dv"], np.float32)
    w_krope = np.asarray(inputs["w_krope"], np.float32)
    w_proj = np.asarray(inputs["w_proj"], np.float32)
    qw = np.asarray(inputs["q_norm_w"], np.float32)
    kvw = np.asarray(inputs["kv_norm_w"], np.float32)

    perm = np.concatenate([np.arange(0, RD, 2), np.arange(1, RD, 2)])

    wkvp = _pack(w_ckv.T.astype(BF))            # [128, 16*512]
    wkrp = _pack(w_krope[perm, :].T.astype(BF))  # [128, 16*64]
    wlqp = _pack(w_cq.T.astype(BF))             # [128, 16*1536]
    wdqn = (w_dq_nope * qw[None, :])          # [H*ND, QR]
    wdqr = (w_dq_rope * qw[None, :]).reshape(H, RD, QR)[:, perm, :]
    wdkn = (w_dk_nope * kvw[None, :])
    wdv = (w_dv * kvw[None, :])
    wprojT = np.ascontiguousarray(w_proj.T).astype(BF)  # [H*HD (o), H*HD (p)]

    csA = np.tile(np.concatenate([fc, fs], axis=1), (1, HC)).astype(BF)  # [S, 256]
    csB = np.tile(np.concatenate([fs, fc], axis=1), (1, HC)).astype(BF)
    maskT = np.zeros((128, 128), np.float32)
    il, jl = np.tril_indices(128, -1)   # sq < sk  -> masked
    maskT[il, jl] = NEG

    csAp, csBp = _pack(csA), _pack(csB)

    in_maps = []
    for c in range(NC):
        b, g = divmod(c, G)
        rows = np.arange(g * SC, (g + 1) * SC)
        hsl = slice(g * HC, (g + 1) * HC)
        xTp_c = _pack(np.ascontiguousarray(x[b].T[:, rows]).astype(BF))
        wdqn_c = _pack(np.ascontiguousarray(
            wdqn.reshape(H, ND, QR)[hsl].reshape(HC * ND, QR).T).astype(BF))
        wdqr_c = _pack(np.ascontiguousarray(
            wdqr[hsl].reshape(HC * RD, QR).T).astype(BF))
        wdkn_c = _pack(np.ascontiguousarray(
            wdkn.reshape(H, ND, KVR)[hsl].reshape(HC * ND, KVR).T).astype(BF))
        wdv_c = _pack(np.ascontiguousarray(
            wdv.reshape(H, HD, KVR)[hsl].reshape(HC * HD, KVR).T).astype(BF))
        in_maps.append({
            "xTp": xTp_c,
            "wkvp": wkvp,
            "wkrp": wkrp,
            "wlqp": wlqp,
            "wdqn": wdqn_c,
            "wdqr": wdqr_c,
            "wdkn": wdkn_c,
            "wdv": wdv_c,
            "wproj": wprojT,
            "csA": csAp,
            "csB": csBp,
            "csAc": _pack(np.ascontiguousarray(csA[rows, :RD])),
            "csBc": _pack(np.ascontiguousarray(csB[rows, :RD])),
            "maskT": maskT,
        })
    return in_maps


def _assemble(results):
    out = np.zeros((B, S, H * HD), np.float32)
    for c in range(NC):
        out[0, 256 * c:256 * (c + 1), :] = results[c]["out"][0:256]
        out[1, 256 * c:256 * (c + 1), :] = results[c]["out"][256:512]
    return out


def kernel(**inputs) -> np.ndarray:
    from concourse.bass_utils import run_bass_kernel_spmd
    nc = _get_nc()
    in_maps = _prep_inputs(inputs)
    res = run_bass_kernel_spmd(nc, in_maps, core_ids=list(range(NC)))
    return _assemble(res.results)
